# revision 38
# baseline (speedup 1.0000x reference)
"""GroupedQueryAttention+RoPE Trainium2 kernel (8 NeuronCores).

Model: d_model=2048, H=32 q-heads, Hkv=8 kv-heads (G=4), head_dim=64,
B=2, T=2048, causal, softmax without max-subtraction (scores are O(6)).

Sharding: 2-way data parallel on batch x 4-way tensor parallel on heads.
Core c: batch = c//4, kv heads {2j, 2j+1} (j=c%4), q heads [8j:8j+8].
Per-core out-proj is row-sharded; host sums the 4 partials per batch.

Device dataflow (per core):
  x loaded once per 512-token chunk in [e,t] layout
  q/k projections in [e,t] (f32r); v projection in [e,t] + PE-transpose
  RoPE via half-swap DMA + DVE muls with host cos/sin maps
  scores^T[tk,tq] pairs of K=64 matmuls (tile_position); the 2 heads of
  a pair share one 2-bank psum tile => one exp per kt-tile; diagonal
  blocks trimmed to valid q-ranges (min width 256 keeps f32r full rate)
  ctx^T via V augmented with ones columns => softmax denominator free;
  ctx evacuated to SBUF so the psum bank recycles across p
  out-proj in bf16 (weights+ctx), row-sharded; host sums partials+bias

Emission is interleaved (proj(t+1) sweeps and outproj(t-1) quarters
between attention p-blocks) because Tile's per-engine instruction
order follows emission order: attention is exp(ACT)-bound, and the
interleave lets PE fill those stretches with projection/out-proj work.
"""
import numpy as np

D = 2048
T = 2048
B = 2
HD = 64
HALF = 32
THETA = 10000.0
NC = 8
TC = 4          # t-chunks of 512
CH = 512        # chunk width
KT = 16         # 128-row k-tiles per projection contraction

_compiled = None
_trace = False
_trace_sim = False
_debug = False
_last = None


def _build():
    import concourse.bacc as bacc
    import concourse.mybir as mybir
    from concourse.tile import TileContext

    F32 = mybir.dt.float32
    F32R = mybir.dt.float32r
    BF16 = mybir.dt.bfloat16
    Exp = mybir.ActivationFunctionType.Exp

    nc = bacc.Bacc("TRN2", target_bir_lowering=False, debug=False, num_devices=NC)

    xT = nc.dram_tensor("xT", [D, T], BF16, kind="ExternalInput")
    wq = nc.dram_tensor("wq", [128, 4, KT, 128], BF16, kind="ExternalInput")
    wk = nc.dram_tensor("wk", [128, KT, 128], BF16, kind="ExternalInput")
    wv = nc.dram_tensor("wv", [128, KT, 128], BF16, kind="ExternalInput")
    wo = nc.dram_tensor("wo", [512, D], BF16, kind="ExternalInput")
    crep_d = nc.dram_tensor("crep", [128, T], F32, kind="ExternalInput")
    srep_d = nc.dram_tensor("srep", [128, T], F32, kind="ExternalInput")
    masks_d = nc.dram_tensor("masks", [128, 2, 640], F32R, kind="ExternalInput")
    iden_d = nc.dram_tensor("iden", [128, 128], F32R, kind="ExternalInput")
    perm_d = nc.dram_tensor("perm", [128, 128], F32R, kind="ExternalInput")
    out = nc.dram_tensor("out", [T, D], BF16, kind="ExternalOutput")
    if _debug:
        xdbg = nc.dram_tensor("xdbg", [128, 4, CH], BF16, kind="ExternalOutput")
        qdbg = nc.dram_tensor("qdbg", [128, 4, CH], F32, kind="ExternalOutput")
        kdbg = nc.dram_tensor("kdbg", [128, CH], F32, kind="ExternalOutput")
        vdbg = nc.dram_tensor("vdbg", [128, 4, 130], F32, kind="ExternalOutput")
        pdbg = nc.dram_tensor("pdbg", [128, 2, CH], F32, kind="ExternalOutput")
        cdbg = nc.dram_tensor("cdbg", [128, 4, CH], BF16, kind="ExternalOutput")
        odbg = nc.dram_tensor("odbg", [128, CH], F32, kind="ExternalOutput")
        cpdbg = nc.dram_tensor("cpdbg", [65, 2, CH], F32, kind="ExternalOutput")
        recdbg = nc.dram_tensor("recdbg", [1, 2, CH], F32, kind="ExternalOutput")
        bcdbg = nc.dram_tensor("bcdbg", [64, 2, CH], F32, kind="ExternalOutput")
        sbdbg = nc.dram_tensor("sbdbg", [64, CH], F32, kind="ExternalOutput")
        c2dbg = nc.dram_tensor("c2dbg", [128, 4, CH], F32, kind="ExternalOutput")
        wdbg = nc.dram_tensor("wdbg", [128, D], F32, kind="ExternalOutput")

    xTr = xT.rearrange("(k p) t -> p k t", p=128)
    wor = wo.rearrange("(g p) n -> p g n", p=128)

    # diagonal-strip trim: r -> (q0, mask col range [m0,m1), mask offset)
    TRIM = {0: (0, 0, 128, 0), 1: (128, 128, 256, 128),
            2: (256, 256, 384, 256), 3: (256, 256, 512, 384)}

    with TileContext(nc, trace_sim=_trace_sim) as tc:
        with (
            tc.tile_pool(name="consts", bufs=1) as consts,
            tc.tile_pool(name="wpool", bufs=1) as wpool,
            tc.tile_pool(name="xpool", bufs=4) as xpool,
            tc.tile_pool(name="tabs", bufs=2) as tabs,
            tc.tile_pool(name="qtpool", bufs=2) as qtpool,
            tc.tile_pool(name="ktpool", bufs=4) as ktpool,
            tc.tile_pool(name="vpool", bufs=4) as vpool,
            tc.tile_pool(name="ctxpool", bufs=2) as ctxpool,
            tc.tile_pool(name="ppool", bufs=4) as ppool,
            tc.tile_pool(name="rpool", bufs=2) as rpool,
            tc.tile_pool(name="opool", bufs=2) as opool,
            tc.tile_pool(name="ps", bufs=2, space="PSUM") as ps,
        ):
            # ---- resident constants / weights ----
            maskst = consts.tile([128, 2, 640], F32R)
            iden = consts.tile([128, 128], F32R)
            ones = consts.tile([128, 1], F32)
            nc.vector.memset(ones, 1.0)
            perm_s = consts.tile([128, 128], F32R)
            wq_s = wpool.tile([128, 4, KT, 128], BF16)
            wk_s = wpool.tile([128, KT, 128], BF16)
            wv_s = wpool.tile([128, KT, 128], BF16)
            wo_s = wpool.tile([128, 4, D], BF16)

            # lead-in: first-needed transfers first, split small so the
            # first sweep starts early. masks/iden/wo are emitted after
            # proj(0) - they are not needed until attention/out-proj.
            _xc0 = []
            for pair in range(2):
                nc.sync.dma_start(out=wq_s[:, pair, 0:8, :],
                                  in_=wq[:, pair, 0:8, :])
            xc = xpool.tile([128, 4, CH], BF16, tag="x", name="xc_0_0")
            nc.sync.dma_start(out=xc, in_=xTr[:, 0:4, 0:CH])
            _xc0.append(xc)
            for pair in range(2):
                nc.sync.dma_start(out=wq_s[:, pair, 8:16, :],
                                  in_=wq[:, pair, 8:16, :])
            xc = xpool.tile([128, 4, CH], BF16, tag="x", name="xc_0_1")
            nc.sync.dma_start(out=xc, in_=xTr[:, 4:8, 0:CH])
            _xc0.append(xc)
            xc = xpool.tile([128, 4, CH], BF16, tag="x", name="xc_0_2")
            nc.sync.dma_start(out=xc, in_=xTr[:, 8:12, 0:CH])
            _xc0.append(xc)
            xc = xpool.tile([128, 4, CH], BF16, tag="x", name="xc_0_3")
            nc.sync.dma_start(out=xc, in_=xTr[:, 12:16, 0:CH])
            _xc0.append(xc)
            crep0 = tabs.tile([128, CH], F32, tag="crep", name="crep_0")
            srep0 = tabs.tile([128, CH], F32, tag="srep", name="srep_0")
            nc.sync.dma_start(out=crep0, in_=crep_d[:, 0:CH])
            nc.sync.dma_start(out=srep0, in_=srep_d[:, 0:CH])
            nc.sync.dma_start(out=perm_s, in_=perm_d[:, :])
            nc.sync.dma_start(out=iden, in_=iden_d[:, :])
            for pair in range(2, 4):
                nc.sync.dma_start(out=wq_s[:, pair, :, :], in_=wq[:, pair, :, :])
            nc.sync.dma_start(out=wk_s, in_=wk[:, :, :])
            nc.sync.dma_start(out=wv_s, in_=wv[:, :, :])
            # warm the ACT Exp table / DVE reciprocal / gpsimd broadcast
            # ucode off the critical path (first-use loads are persistent
            # device state; racing them poisons the first execution)
            warm = rpool.tile([1, 1], F32R, tag="warm", bufs=1)
            nc.scalar.activation(warm, ones[0:1, 0:1], Exp, scale=0.125)
            warm2 = rpool.tile([1, 1], F32, tag="warm2", bufs=1)
            nc.vector.reciprocal(warm2, ones[0:1, 0:1])
            warm3 = rpool.tile([64, 1], F32, tag="warm3", bufs=1)
            nc.gpsimd.partition_broadcast(warm3, warm2[0:1, :])

            kts = []     # kTt tiles per tchunk (persist)
            vts = []     # v_augt tiles per tchunk (persist)
            qT_all = []  # per-tcix q tiles
            ctxqs = []   # per-tcix normalized ctx tiles
            xcs = {0: _xc0}
            tables = {0: (crep0, srep0)}
            state = {}   # per-tcix proj tiles

            def rope(dst, psrc, crep, srep, tag, on_dve=False, ps2=None):
                """dst (f32r sbuf) = rope(psrc [128,CH] psum); the half-swap
                is a PE permutation matmul (saves two DMA round-trips).
                The psum-releasing copy goes on ACT or DVE so the two
                ropes of a sweep release their slots concurrently; the
                two ropes of a sweep share one ps2 slot (they serialize
                on DVE anyway) so a proj slot frees one rope earlier."""
                qs = rpool.tile([128, CH], F32R, tag="qs", name=f"qs_{tag}")
                if on_dve:
                    nc.vector.tensor_copy(qs, psrc[:, :])
                else:
                    nc.scalar.copy(qs, psrc[:, :])
                if ps2 is None:
                    ps2 = ps.tile([128, CH], F32, tag="proj", name=f"ps2_{tag}")
                nc.tensor.matmul(ps2[:, :], perm_s, qs, start=True, stop=True)
                nc.vector.tensor_mul(dst, qs, crep)
                t2 = rpool.tile([128, CH], F32, tag="qsh", name=f"t2_{tag}")
                nc.vector.tensor_mul(t2, ps2[:, :], srep)
                nc.vector.tensor_add(dst, dst, t2)

            def proj_sweep(tcix, sweep):
                """one projection sweep for t-chunk tcix.
                sweep 0: q pairs {0,1}; 1: q pairs {2,3}; 2: {k, v}."""
                tsl = slice(tcix * CH, (tcix + 1) * CH)
                if sweep == 0:
                    if tcix not in xcs:
                        lst = []
                        for c in range(4):
                            xc = xpool.tile([128, 4, CH], BF16, tag="x",
                                            name=f"xc_{tcix}_{c}")
                            nc.sync.dma_start(out=xc,
                                              in_=xTr[:, 4 * c:4 * c + 4, tsl])
                            lst.append(xc)
                        xcs[tcix] = lst
                    if tcix not in tables:
                        crep = tabs.tile([128, CH], F32, tag="crep",
                                         name=f"crep_{tcix}")
                        srep = tabs.tile([128, CH], F32, tag="srep",
                                         name=f"srep_{tcix}")
                        nc.sync.dma_start(out=crep, in_=crep_d[:, tsl])
                        nc.sync.dma_start(out=srep, in_=srep_d[:, tsl])
                        tables[tcix] = (crep, srep)
                    qTt = qtpool.tile([128, 4, CH], F32R, tag="qTt",
                                      name=f"qTt_{tcix}")
                    kTt = ktpool.tile([128, CH], F32R, tag="kTt",
                                      name=f"kTt_{tcix}")
                    v_augt = vpool.tile([128, 4, 130], F32R, tag="vat",
                                        name=f"vat_{tcix}")
                    for vt in range(4):
                        nc.vector.tensor_copy(v_augt[:, vt, 64:65], ones)
                        nc.vector.tensor_copy(v_augt[:, vt, 129:130], ones)
                    if _debug and tcix == 0:
                        nc.sync.dma_start(out=xdbg[:, :, :], in_=xcs[0][0])
                        qf = rpool.tile([128, 4, CH], F32, tag="dbg", bufs=1)
                        nc.vector.tensor_copy(qf, qTt)
                        nc.sync.dma_start(out=qdbg[:, :, :], in_=qf)
                        kf = rpool.tile([128, CH], F32, tag="dbg", bufs=1)
                        nc.vector.tensor_copy(kf, kTt)
                        nc.sync.dma_start(out=kdbg[:, :], in_=kf)
                        vf = rpool.tile([128, 4, 130], F32, tag="dbg", bufs=1)
                        nc.vector.tensor_copy(vf, v_augt)
                        nc.sync.dma_start(out=vdbg[:, :, :], in_=vf)
                    state[tcix] = (qTt, kTt, v_augt)
                    qT_all.append(qTt)
                    kts.append(kTt)
                    vts.append(v_augt)
                crep, srep = tables[tcix]
                qTt, kTt, v_augt = state[tcix]
                xc = xcs[tcix]

                psL = ps.tile([128, CH], F32, tag="proj", name=f"psL_{tcix}_{sweep}")
                psR = ps.tile([128, CH], F32, tag="proj", name=f"psR_{tcix}_{sweep}")
                qsweep = {0: (0, 1), 1: (2, 3)}
                for k in range(KT):
                    st, sp = (k == 0), (k == KT - 1)
                    xk = xc[k // 4][:, k % 4, :]
                    if sweep != 2:
                        pa, pb = qsweep[sweep]
                        nc.tensor.matmul(psL[:, :], wq_s[:, pa, k, :],
                                         xk, start=st, stop=sp)
                        nc.tensor.matmul(psR[:, :], wq_s[:, pb, k, :],
                                         xk, start=st, stop=sp)
                    else:
                        nc.tensor.matmul(psL[:, :], wk_s[:, k, :], xk,
                                         start=st, stop=sp)
                        nc.tensor.matmul(psR[:, :], wv_s[:, k, :], xk,
                                         start=st, stop=sp)
                if sweep != 2:
                    pa, pb = qsweep[sweep]
                    ps2s = ps.tile([128, CH], F32, tag="proj",
                                   name=f"ps2s_{tcix}_{sweep}")
                    rope(qTt[:, pa, :], psL, crep, srep, f"q{pa}_{tcix}",
                         ps2=ps2s)
                    rope(qTt[:, pb, :], psR, crep, srep, f"q{pb}_{tcix}",
                         on_dve=True, ps2=ps2s)
                else:
                    rope(kTt[:, :], psL, crep, srep, f"k_{tcix}")
                    # v: [e,t] -> PE transpose -> [t,e] augmented with ones
                    vfT = rpool.tile([128, CH], F32R, tag="vfT", bufs=1,
                                     name=f"vfT_{tcix}")
                    nc.scalar.copy(vfT, psR[:, :])
                    vtp = ps.tile([128, 4, 128], F32R, tag="proj",
                                  name=f"vtp_{tcix}")
                    for vt in range(4):
                        nc.tensor.transpose(vtp[:, vt, :],
                                            vfT[:, vt * 128:(vt + 1) * 128],
                                            iden)
                    for vt in range(4):
                        nc.scalar.copy(v_augt[:, vt, 0:64], vtp[:, vt, 0:64])
                        nc.scalar.copy(v_augt[:, vt, 65:129],
                                       vtp[:, vt, 64:128])

            def attn_p(tcix, p):
                """attention p-block for q-chunk tcix over k-chunks 0..tcix."""
                qTt = qT_all[tcix]
                nkt = 4 * tcix + 4
                if p == 0:
                    ctxq = ctxpool.tile([128, 4, CH], BF16, tag="ctxq",
                                        name=f"ctxq_{tcix}")
                    ctxqs.append(ctxq)
                ctxq = ctxqs[tcix]
                ctx2 = ps.tile([128, 2, CH], F32, tag="ctx", bufs=1,
                               name=f"ctx2_{tcix}_{p}")
                for kt in range(nkt):
                    ktile = kts[kt // 4]
                    vtile = vts[kt // 4]
                    ksl = slice((kt % 4) * 128, (kt % 4 + 1) * 128)
                    r = kt - 4 * tcix
                    q0 = TRIM[r][0] if r >= 0 else 0
                    sp2 = ps.tile([128, 2, CH], F32, tag="sp",
                                  name=f"sp2_{tcix}_{p}_{kt}")
                    nc.tensor.matmul(sp2[:, 0, q0:CH], ktile[0:64, ksl],
                                     qTt[0:64, p, q0:CH],
                                     start=True, stop=True, tile_position=(0, 0))
                    nc.tensor.matmul(sp2[:, 1, q0:CH], ktile[64:128, ksl],
                                     qTt[64:128, p, q0:CH],
                                     start=True, stop=True, tile_position=(64, 0))
                    pAB = ppool.tile([128, 2, CH], F32R, tag="p",
                                     name=f"p_{tcix}_{p}_{kt}")
                    nc.scalar.activation(pAB[:, :, q0:CH], sp2[:, :, q0:CH],
                                         Exp, scale=0.125)
                    if r >= 0:
                        _, m0, m1, moff = TRIM[r]
                        nc.vector.tensor_mul(pAB[:, :, m0:m1],
                                             pAB[:, :, m0:m1],
                                             maskst[:, :, moff:moff + m1 - m0])
                    if _debug and tcix == 0 and p == 0 and kt == 0:
                        pf = rpool.tile([128, 2, CH], F32, tag="dbg", bufs=1)
                        nc.vector.tensor_copy(pf, pAB)
                        nc.sync.dma_start(out=pdbg[:, :, :], in_=pf)
                    st, sp = (kt == 0), (kt == nkt - 1)
                    nc.tensor.matmul(ctx2[0:65, 0, q0:CH],
                                     vtile[:, kt % 4, 0:65],
                                     pAB[:, 0, q0:CH], start=st, stop=sp)
                    nc.tensor.matmul(ctx2[0:65, 1, q0:CH],
                                     vtile[:, kt % 4, 65:130],
                                     pAB[:, 1, q0:CH], start=st, stop=sp)
                # evacuate ctx (frees the psum bank), then normalize:
                # rows 0-63 ctx, row 64 denominator. For the final p the
                # evacuation is skipped (nothing reuses the bank) to cut
                # the kernel-tail serial chain.
                if tcix == 3 and p == 3:
                    cp = ctx2
                else:
                    cp = rpool.tile([65, 2, CH], F32, tag="cp",
                                    name=f"cp_{tcix}_{p}")
                    nc.vector.tensor_copy(cp[0:65, :, :], ctx2[0:65, :, :])
                rec = rpool.tile([1, 2, CH], F32, tag="rec",
                                 name=f"rec_{tcix}_{p}")
                nc.vector.reciprocal(rec[0:1, 0, :], cp[64:65, 0, :])
                nc.vector.reciprocal(rec[0:1, 1, :], cp[64:65, 1, :])
                bc2 = rpool.tile([64, 2, CH], F32, tag="bc",
                                 name=f"bc_{tcix}_{p}")
                nc.gpsimd.partition_broadcast(bc2[:, 0, :], rec[0:1, 0, :])
                nc.gpsimd.partition_broadcast(bc2[:, 1, :], rec[0:1, 1, :])
                nc.vector.tensor_mul(ctxq[0:64, p, :], cp[0:64, 0, :],
                                     bc2[:, 0, :])
                stB = rpool.tile([64, CH], BF16, tag="stB",
                                 name=f"stB_{tcix}_{p}")
                nc.vector.tensor_mul(stB, cp[0:64, 1, :], bc2[:, 1, :])
                nc.sync.dma_start(out=ctxq[64:128, p, :], in_=stB)
                if _debug and tcix == 0 and p == 0:
                    cpf = rpool.tile([65, 2, CH], F32, tag="dbg", bufs=1)
                    nc.vector.tensor_copy(cpf, cp)
                    nc.sync.dma_start(out=cpdbg[:, :, :], in_=cpf)
                    recf = rpool.tile([1, 2, CH], F32, tag="dbg", bufs=1)
                    nc.vector.tensor_copy(recf, rec)
                    nc.sync.dma_start(out=recdbg[:, :, :], in_=recf)
                    bcf = rpool.tile([64, 2, CH], F32, tag="dbg", bufs=1)
                    nc.vector.tensor_copy(bcf, bc2)
                    nc.sync.dma_start(out=bcdbg[:, :, :], in_=bcf)
                    sbf = rpool.tile([64, CH], F32, tag="dbg", bufs=1)
                    nc.vector.tensor_copy(sbf, stB)
                    nc.sync.dma_start(out=sbdbg[:, :], in_=sbf)
                if _debug and tcix == 0 and p == 3:
                    nc.sync.dma_start(out=cdbg[:, :, :], in_=ctxq)

            def outproj_q(tcix, i):
                """out-projection for token block i of t-chunk tcix."""
                ctxq = ctxqs[tcix]
                tt = 4 * tcix + i
                use_sp = False
                if _debug and tcix == 0 and i == 0:
                    c2f = rpool.tile([128, 4, CH], F32, tag="dbg", bufs=1)
                    nc.vector.tensor_copy(c2f, ctxq)
                    nc.sync.dma_start(out=c2dbg[:, :, :], in_=c2f)
                    wf = rpool.tile([128, D], F32, tag="dbg", bufs=1)
                    nc.vector.tensor_copy(wf, wo_s[:, 0, :])
                    nc.sync.dma_start(out=wdbg[:, :], in_=wf)
                for half in range(2):
                    ost = opool.tile([128, 1024], BF16, tag="ost",
                                     name=f"ost_{tt}_{half}")
                    if use_sp:
                        ops2 = ps.tile([128, 2, CH], F32, tag="sp",
                                       name=f"ops2_{tt}_{half}")
                    for dmh in range(2):
                        dm = 2 * half + dmh
                        if use_sp:
                            ops = ops2[:, dmh, :]
                        else:
                            ops = ps.tile([128, CH], F32, tag="proj",
                                          name=f"ops_{tt}_{dm}")
                        for g in range(4):
                            nc.tensor.matmul(ops[:, :],
                                             ctxq[:, g, i * 128:(i + 1) * 128],
                                             wo_s[:, g, dm * CH:(dm + 1) * CH],
                                             start=(g == 0), stop=(g == 3))
                        if _debug and tt == 0 and dm == 0:
                            of = rpool.tile([128, CH], F32, tag="dbg", bufs=1)
                            nc.vector.tensor_copy(of, ops[:, :])
                            nc.sync.dma_start(out=odbg[:, :], in_=of)
                        if tcix < 2 and dm % 2 == 1:
                            nc.scalar.copy(
                                ost[:, dmh * CH:(dmh + 1) * CH], ops[:, :])
                        else:
                            nc.vector.tensor_copy(
                                ost[:, dmh * CH:(dmh + 1) * CH], ops[:, :])
                    nc.sync.dma_start(
                        out=out[tt * 128:(tt + 1) * 128,
                                half * 1024:(half + 1) * 1024],
                        in_=ost)

            # ---- emission schedule ----
            for s in range(3):
                proj_sweep(0, s)
            # deferred weight/constant loads (needed for attn / out-proj)
            nc.sync.dma_start(out=maskst, in_=masks_d[:, :, :])
            for g in range(4):
                nc.sync.dma_start(out=wo_s[:, g, :], in_=wor[:, g, :])

            for t in range(TC):
                for p in range(4):
                    attn_p(t, p)
                    if t < 3 and p < 3:
                        proj_sweep(t + 1, p)
                    # hold back the last two outproj(2) quarters so their
                    # PE work lands in the attn(3) p3 normalize drain
                    if t >= 1 and (t < 3 or p < 2):
                        outproj_q(t - 1, p)
            outproj_q(2, 2)
            outproj_q(2, 3)
            for i in range(4):
                outproj_q(3, i)

    nc.compile()
    return nc


def _host_tables():
    inv_freq = 1.0 / (THETA ** (np.arange(0, HD, 2, dtype=np.float64) / HD))
    t = np.arange(T, dtype=np.float64)
    freqs = np.outer(t, inv_freq)          # (T, 32)
    cos = np.cos(freqs)
    sin = np.sin(freqs)
    crep = np.empty((128, T), np.float32)
    srep = np.empty((128, T), np.float32)
    for blk in range(4):                   # 4 blocks of 32 partitions
        j = np.arange(HALF)
        crep[blk * 32:(blk + 1) * 32] = cos[:, j].T
        sgn = -1.0 if (blk % 2 == 0) else 1.0
        srep[blk * 32:(blk + 1) * 32] = sgn * sin[:, j].T
    # packed diagonal masks [128, 2, 640]: r windows at offsets 0/128/256/384
    masks = np.zeros((128, 2, 640), np.float32)
    tk = np.arange(128)[:, None]
    wins = [(0, 0, 128, 0), (128, 128, 256, 128),
            (256, 256, 384, 256), (384, 256, 512, 384)]
    for shift, m0, m1, moff in wins:
        qt = np.arange(m0, m1)[None, :]
        masks[:, 0, moff:moff + m1 - m0] = (tk + shift <= qt)
        masks[:, 1, moff:moff + m1 - m0] = masks[:, 0, moff:moff + m1 - m0]
    return crep, srep, masks


def kernel(x, Wq, Wk, Wv, Wo, b_out):
    global _compiled
    import ml_dtypes
    from concourse.bass_utils import run_bass_kernel_spmd

    x = np.asarray(x, np.float32)
    Wq = np.asarray(Wq, np.float32)
    Wk = np.asarray(Wk, np.float32)
    Wv = np.asarray(Wv, np.float32)
    Wo = np.asarray(Wo, np.float32)
    b_out = np.asarray(b_out, np.float32)

    crep, srep, masks = _host_tables()
    iden = np.eye(128, dtype=np.float32)
    perm = np.zeros((128, 128), np.float32)
    for m in range(128):
        blk, j = m // 64, m % 64
        swap = blk * 64 + (j + HALF if j < HALF else j - HALF)
        perm[swap, m] = 1.0

    in_maps = []
    for c in range(NC):
        b, j = c // 4, c % 4
        # q-head pair layout: pair p = (head 8j+p, head 8j+p+4)
        qcols = []
        for p in range(4):
            qcols.append(Wq[:, 64 * (8 * j + p):64 * (8 * j + p) + 64])
            qcols.append(Wq[:, 64 * (8 * j + p + 4):64 * (8 * j + p + 4) + 64])
        wq_c = np.concatenate(qcols, axis=1).astype(ml_dtypes.bfloat16)
        wq_c = np.ascontiguousarray(              # [p, pair, k, c]
            wq_c.reshape(KT, 128, 4, 128).transpose(1, 2, 0, 3))
        wk_c = Wk[:, 128 * j:128 * (j + 1)].astype(ml_dtypes.bfloat16)
        wk_c = np.ascontiguousarray(wk_c.reshape(KT, 128, 128).transpose(1, 0, 2))
        wv_c = Wv[:, 128 * j:128 * (j + 1)].astype(ml_dtypes.bfloat16)
        wv_c = np.ascontiguousarray(wv_c.reshape(KT, 128, 128).transpose(1, 0, 2))
        worows = []
        for g in range(4):
            worows.append(Wo[64 * (8 * j + g):64 * (8 * j + g) + 64, :])
            worows.append(Wo[64 * (8 * j + g + 4):64 * (8 * j + g + 4) + 64, :])
        wo_c = np.ascontiguousarray(
            np.concatenate(worows, axis=0).astype(ml_dtypes.bfloat16))
        in_maps.append({
            "xT": np.ascontiguousarray(x[b].T.astype(ml_dtypes.bfloat16)),
            "wq": wq_c, "wk": wk_c, "wv": wv_c, "wo": wo_c,
            "crep": crep, "srep": srep, "masks": masks, "iden": iden,
            "perm": perm,
        })

    if _compiled is None:
        _compiled = _build()

    global _last
    res = run_bass_kernel_spmd(_compiled, in_maps, core_ids=list(range(NC)),
                               trace=_trace)
    _last = res

    full = np.empty((B, T, D), np.float32)
    for b in range(B):
        acc = res.results[4 * b + 0]["out"].astype(np.float32)
        for j in range(1, 4):
            acc = acc + res.results[4 * b + j]["out"]
        full[b] = acc + b_out[None, :]
    return full
